# revision 3
# baseline (speedup 1.0000x reference)
"""D3BJ dispersion-energy kernel for 8 Trainium2 NeuronCores.

Strategy (data-parallel over atoms, per the sharding hint):
  - Host packs a per-atom record table [x,y,z (bohr), t=sqrt(sqrt(3)*r4r2[Z]),
    sw=alpha/c6, ainv=1/alpha]; the full table is replicated to all 8 cores.
  - Each core owns 12500 atom rows (padded to 12544 = 128*98). The 64
    j-records per row are assembled on the host (this toolchain compiles with
    the vector_dynamic_offsets DGE level disabled, so multi-index indirect
    DMA gathers mis-pair offsets with destination rows; verified empirically)
    and streamed to the device per 7-tile group, double-buffered against the
    compute pipeline.
  - Per-pair math runs on DVE (custom fused ops) + ACT; per-group partial sums
    accumulate via the DVE accumulate path; the host reduces the [128, 14]
    per-core partials in float64 and applies the -27.211368 Hartree->eV scale.
  - Row padding points i-records at a ghost atom and j-indices at a second
    ghost ~10^4 Ang away, making pad-pair contributions ~1e-27 (negligible)
    without any masking.
"""
import sys
import numpy as np

sys.path.insert(0, "/opt/trn_rl_repo")

P = 128
NT = 98            # row tiles per core
M = 64             # neighbors
T = 7              # row tiles per group (one gather + compute unit)
NG = NT // T       # 14 groups
F = T * M          # 448 pair elements per group instruction
RPC = 12500        # real rows per core
RPAD = P * NT      # 12544 padded rows per core
TBL = 100352       # padded table rows
A1c, A2c = 0.5299, 4.6
ANG2BOHR = 1.889716
H2EV = 27.211368

_STATE = {}


def _register_ops():
    from operator import add as op_add
    from concourse import dve_ops
    from concourse.dve_ops import DveOp, OPS
    from concourse.dve_spec import Spec, Src0, Src1, C0, C1, C2, lower, sq, _has_src1
    from concourse.dve_uop import DveOpSpec

    def reg(name, spec):
        if name in dve_ops._SUB_OPCODE_FOR_NAME:
            for o in OPS:
                if o.name == name:
                    return o
        row = dve_ops._CUSTOM_DVE_ROW_BASE + len(OPS)
        assert row < 0x20
        dve_ops._SUB_OPCODE_FOR_NAME[name] = row
        shas = {}
        for ver in ("v3", "v4"):
            sl = DveOpSpec(name=name, opcode=row, uops=lower(spec, ver=ver),
                           rd1_en=_has_src1(spec))
            shas[ver] = sl.sha(ver)
        op = DveOp(name, spec, subdim=False, uops_sha=shas)
        OPS.append(op)
        return op

    ops = {}
    ops["SQSUM2"] = reg("D3_SQSUM2", Spec(
        body=sq(Src0) + sq(Src1),
        reference=lambda in0, in1, s0, s1, imm2: in0 * in0 + in1 * in1))
    ops["ADDSQ"] = reg("D3_ADDSQ", Spec(
        body=Src0 + sq(Src1),
        reference=lambda in0, in1, s0, s1, imm2: in0 + in1 * in1))
    ops["DEN6"] = reg("D3_DEN6", Spec(
        body=sq(Src0) * Src0 + sq(Src1) * Src1,
        reference=lambda in0, in1, s0, s1, imm2: in0 ** 3 + in1 ** 3))
    ops["DEN8"] = reg("D3_DEN8", Spec(
        body=sq(sq(Src0)) + sq(sq(Src1)),
        reference=lambda in0, in1, s0, s1, imm2: in0 ** 4 + in1 ** 4))
    ops["T8"] = reg("D3_T8", Spec(
        body=sq(Src0) * Src1 * C2,
        reference=lambda in0, in1, s0, s1, imm2: in0 * in0 * in1 * imm2))
    ops["MULACC"] = reg("D3_MULACC", Spec(
        body=Src0 * Src1, accum=op_add,
        reference=lambda in0, in1, s0, s1, imm2: in0 * in1))
    return ops


def _build():
    if "nc" in _STATE:
        return _STATE["nc"]
    from concourse import bass, bacc, mybir, tile

    ops = _register_ops()
    SQSUM2, ADDSQ, DEN6, DEN8, T8, MULACC = (ops[k] for k in
        ("SQSUM2", "ADDSQ", "DEN6", "DEN8", "T8", "MULACC"))
    FT = mybir.ActivationFunctionType

    nc = bacc.Bacc("TRN2", target_bir_lowering=False, debug=False,
                   enable_asserts=False, num_devices=8)
    grec_in = nc.dram_tensor("grec", [P, NT * M * 6], mybir.dt.float32, kind="ExternalInput").ap()
    irec_in = nc.dram_tensor("irec", [P, NT * 8], mybir.dt.float32, kind="ExternalInput").ap()
    niter_in = nc.dram_tensor("niter", [1, 1], mybir.dt.int32, kind="ExternalInput").ap()
    accA_out = nc.dram_tensor("accA", [P, NG], mybir.dt.float32, kind="ExternalOutput").ap()
    accB_out = nc.dram_tensor("accB", [P, NG], mybir.dt.float32, kind="ExternalOutput").ap()

    with tile.TileContext(nc) as tc:
        with tc.tile_pool(name="persist", bufs=1) as persist, \
             tc.tile_pool(name="g6", bufs=3) as g6pool, \
             tc.tile_pool(name="work", bufs=2) as wp, \
             tc.tile_pool(name="acc", bufs=1) as accp:
            irec = persist.tile([P, NT * 8], mybir.dt.float32)
            nc.sync.dma_start(out=irec[:], in_=irec_in[:])
            nit = persist.tile([1, 1], mybir.dt.int32)
            nc.sync.dma_start(out=nit[:], in_=niter_in[:])
            nval = nc.values_load(nit[0:1, 0:1], min_val=0, max_val=100000,
                                  skip_runtime_bounds_check=True)
            a2t = persist.tile([P, 1], mybir.dt.float32)
            nc.gpsimd.memset(a2t[:], A2c)
            accA = accp.tile([P, NG], mybir.dt.float32)
            accB = accp.tile([P, NG], mybir.dt.float32)

            def compute_group(g, gsrc):
                gs = gsrc.rearrange("p (f a) -> p a f", a=6)
                xj, yj, zj, tj, swj, aij = (gs[:, a] for a in range(6))
                iv = irec[:, g * T * 8:(g + 1) * T * 8].rearrange("p (t a) -> p a t", a=8)
                ib = lambda a: iv[:, a][:, :, None].to_broadcast([P, T, M])
                xib, yib, zib, tib, swib, aiib = (ib(a) for a in range(6))
                wt = lambda tag: wp.tile([P, F], mybir.dt.float32, tag=tag, name=tag)
                v3 = lambda ap: ap.rearrange("p (t m) -> p t m", m=M)
                dx, dy, dz = wt("sA"), wt("sB"), wt("sC")
                nc.vector.tensor_tensor(out=v3(dx[:]), in0=v3(xj), in1=xib, op=mybir.AluOpType.subtract)
                nc.vector.tensor_tensor(out=v3(dy[:]), in0=v3(yj), in1=yib, op=mybir.AluOpType.subtract)
                nc.vector.tensor_tensor(out=v3(dz[:]), in0=v3(zj), in1=zib, op=mybir.AluOpType.subtract)
                m2 = wt("sD")
                nc.vector._custom_dve(SQSUM2, out=m2[:], in0=dx[:], in1=dy[:])
                d2 = wt("sA")
                nc.vector._custom_dve(ADDSQ, out=d2[:], in0=m2[:], in1=dz[:])
                st = wt("sB")
                nc.vector.tensor_tensor(out=v3(st[:]), in0=v3(tj), in1=tib, op=mybir.AluOpType.mult)
                r02 = wt("sC")
                nc.scalar.activation(r02[:], st[:], FT.Square, bias=a2t[:], scale=A1c)
                den6, den8 = wt("sD"), wt("sE")
                nc.vector._custom_dve(DEN6, out=den6[:], in0=d2[:], in1=r02[:])
                nc.vector._custom_dve(DEN8, out=den8[:], in0=d2[:], in1=r02[:])
                inv6, inv8 = wt("sC"), wt("sD")
                nc.vector.reciprocal_approx_fast(out=inv6[:], in_=den6[:])
                nc.vector.reciprocal_approx_fast(out=inv8[:], in_=den8[:])
                t8 = wt("sE")
                nc.vector._custom_dve(T8, out=t8[:], in0=st[:], in1=inv8[:], imm2=2.0)
                mA, mB = wt("sA"), wt("sF")
                nc.vector.tensor_tensor(out=v3(mA[:]), in0=v3(swj), in1=aiib, op=mybir.AluOpType.mult)
                nc.vector.tensor_tensor(out=v3(mB[:]), in0=v3(aij), in1=swib, op=mybir.AluOpType.mult)
                denc = wt("sB")
                nc.vector.tensor_tensor(out=denc[:], in0=mA[:], in1=mB[:], op=mybir.AluOpType.add)
                invc = wt("sF")
                nc.vector.reciprocal_approx_fast(out=invc[:], in_=denc[:])
                sc1, sc2 = wt("sA"), wt("sD")
                nc.vector._custom_dve(MULACC, out=sc1[:], in0=inv6[:], in1=invc[:], accum_out=accA[:, g:g + 1])
                nc.vector._custom_dve(MULACC, out=sc2[:], in0=t8[:], in1=invc[:], accum_out=accB[:, g:g + 1])

            with tc.For_i(0, nval) as _:
                for g in range(NG):
                    gt = g6pool.tile([P, F * 6], mybir.dt.float32, tag="g6", name="g6t")
                    nc.sync.dma_start(out=gt[:], in_=grec_in[:, g * F * 6:(g + 1) * F * 6])
                    compute_group(g, gt[:])

            nc.sync.dma_start(out=accA_out[:], in_=accA[:])
            nc.sync.dma_start(out=accB_out[:], in_=accB[:])

    nc.compile()
    _STATE["nc"] = nc
    return nc


def _prep_inputs(coord, c6, alpha, r4r2, idx_j, numbers):
    coord = np.asarray(coord, np.float32)
    c6 = np.asarray(c6, np.float32)
    alpha = np.asarray(alpha, np.float32)
    r4r2 = np.asarray(r4r2, np.float32)
    idx_j = np.asarray(idx_j, np.int32)
    numbers = np.asarray(numbers, np.int32)
    N = coord.shape[0]

    u = np.float32(np.sqrt(3.0)) * r4r2[numbers]
    table6 = np.zeros((TBL, 6), np.float32)
    table6[:N, 0:3] = coord * np.float32(ANG2BOHR)
    table6[:N, 3] = np.sqrt(u)
    table6[:N, 4] = alpha / c6
    table6[:N, 5] = np.float32(1.0) / alpha
    GH = np.float32(3000.0 * ANG2BOHR)
    table6[TBL - 2] = [GH, GH, GH, 1.0, 1.0, 1.0]
    table6[TBL - 1] = [-GH, -GH, -GH, 1.0, 1.0, 1.0]

    idx_pad = np.full((8, RPAD, M), TBL - 1, np.int32)
    irec_pad = np.zeros((8, RPAD, 8), np.float32)
    irec_pad[:, :, 0:6] = table6[TBL - 2]
    for c in range(8):
        idx_pad[c, :RPC] = idx_j[c * RPC:(c + 1) * RPC]
        irec_pad[c, :RPC, 0:6] = table6[c * RPC:(c + 1) * RPC]
    # j-record gather on host (device-side indirect DMA gathers are broken in
    # this toolchain: vector_dynamic_offsets DGE level is compiler-disabled)
    grec = table6[idx_pad.reshape(8, -1)].reshape(8, P, NT * M * 6)
    return grec, idx_pad.reshape(8, P, NT * M), irec_pad.reshape(8, P, NT * 8)


def _run(grec, idx_maps, irec_maps, niter=1):
    from concourse import bass_utils
    nc = _build()
    nv = np.array([[niter]], np.int32)
    ins = [{"grec": grec[c], "irec": irec_maps[c], "niter": nv} for c in range(8)]
    return bass_utils.run_bass_kernel_spmd(nc, ins, core_ids=list(range(8)))


def kernel(coord, c6, alpha, r4r2, idx_j, numbers, nb_pad_mask=None):
    grec, idx_maps, irec_maps = _prep_inputs(coord, c6, alpha, r4r2, idx_j, numbers)
    res = _run(grec, idx_maps, irec_maps, niter=1)
    tot = np.float64(0.0)
    for c in range(8):
        tot += np.float64(res.results[c]["accA"]).sum()
        tot += np.float64(res.results[c]["accB"]).sum()
    return np.float32(-H2EV * tot)
